# revision 16
# baseline (speedup 1.0000x reference)
"""Trainium2 Bass kernel: GQA attention (B=2, S=2048, D=2048, 32 q-heads,
8 kv-heads, head_dim 64, RoPE interleaved, causal) on 8 NeuronCores.

Sharding: tensor-parallel over heads. Core c owns q-heads 4c..4c+3 (= kv head
c) for BOTH batch elements. The kernel streams x one (batch, half) chunk at a
time, projecting q/k/v for that chunk, and INTERLEAVES causal-attention block
work for already-projected query groups between the projection passes so the
PE never idles (idle PE re-throttles the HAM clock gate to 1.2 GHz). Query
groups are contiguous 512-token quarters; group (b,u) needs key blocks
0..4u+3, so groups become runnable as soon as their (batch, half) chunks are
projected. Attention outputs for u in {0,1} ship through an 8-core AllToAll
mid-kernel while u in {2,3} attention still runs; the output projection for
the first token-halves overlaps the remaining attention, and only the second
AllToAll + final 16 matmul chains sit in the tail.

All matmul inputs are bf16 (PSUM accumulation stays fp32). exp runs on
ScalarE with the 1/sqrt(hd) scale folded into the activation's affine
pre-scale. Softmax denominators come from a ones-column accumulated alongside
attn@V; normalization broadcasts the raw sum with a K=1 matmul FIRST, then
reciprocals the [64,512] broadcast (a [1,512] reciprocal serializes one DVE
lane at 3.3us).
"""

import numpy as np

B, S, D = 2, 2048, 2048
NH, NKV, HD = 32, 8, 64
THETA = 10000.0
NCORES = 8
NEG = -1.0e30

_BUILT = None


def _swap_mask():
    m = []
    for i in range(16):
        m += [2 * i + 1, 2 * i]
    return m


def _build():
    """Build + compile the SPMD Bass program (once per process)."""
    global _BUILT
    if _BUILT is not None:
        return _BUILT

    from contextlib import ExitStack

    import concourse.tile as tile
    from concourse import bacc, mybir
    from concourse.masks import make_identity

    f32 = mybir.dt.float32
    bf = mybir.dt.bfloat16
    AF = mybir.ActivationFunctionType

    nc = bacc.Bacc(
        "TRN2", target_bir_lowering=False, debug=False, num_devices=NCORES
    )

    xT = nc.dram_tensor("xT", [B, 16, 2, 128, 1024], bf, kind="ExternalInput").ap()
    wqTc = nc.dram_tensor("wqTc", [D, 256], bf, kind="ExternalInput").ap()
    wkvTc = nc.dram_tensor("wkvTc", [D, 128], bf, kind="ExternalInput").ap()
    woT = nc.dram_tensor("woT", [D, D], bf, kind="ExternalInput").ap()
    cosd = nc.dram_tensor("cosd", [128, S], bf, kind="ExternalInput").ap()
    sind = nc.dram_tensor("sind", [128, S], bf, kind="ExternalInput").ap()
    maskd = nc.dram_tensor("maskd", [4, 128, 512], bf, kind="ExternalInput").ap()
    onesd = nc.dram_tensor("onesd", [128, 64], bf, kind="ExternalInput").ap()
    outT = nc.dram_tensor("outT", [D, 512], f32, kind="ExternalOutput").ap()

    SW = _swap_mask()
    SCALE = 1.0 / float(np.sqrt(HD))

    with tile.TileContext(nc) as tc, ExitStack() as top:
        top.enter_context(
            nc.allow_low_precision(reason="bf16 matmul inputs by design")
        )
        res = top.enter_context(tc.tile_pool(name="resident", bufs=1))
        qt = [res.tile([128, B * S], bf, tag=f"qt{p}", name=f"qt{p}") for p in range(2)]
        kt = res.tile([128, B * S], bf, tag="kt")  # kv head, duplicated rows
        vt = [res.tile([128, HD + 1], bf, tag=f"vt{i}", name=f"vt{i}") for i in range(2 * 16)]
        msk = [res.tile([128, 512], bf, tag=f"m{j}", name=f"m{j}") for j in range(4)]
        ones_t = res.tile([128, 64], bf, tag="ones")
        cos_t = res.tile([128, B * S], bf, tag="cos")
        sin_t = res.tile([128, B * S], bf, tag="sin")
        ident = res.tile([128, 128], bf, tag="ident")
        wq_t = [res.tile([128, 256], bf, tag=f"wq{d}", name=f"wq{d}") for d in range(16)]
        wkv_t = [res.tile([128, 128], bf, tag=f"wkv{d}", name=f"wkv{d}") for d in range(16)]
        wo_t = [res.tile([128, D], bf, tag=f"wo{e}", name=f"wo{e}") for e in range(16)]
        rh_t = [
            [res.tile([128, 256], bf, tag=f"rh{h}{e}", name=f"rh{h}{e}") for e in range(16)]
            for h in range(2)
        ]

        # weight + first-chunk loads first so the first matmul starts early
        nc.sync.dma_start(out=ones_t[:], in_=onesd[:])
        nc.sync.dma_start(out=wkv_t[0][:], in_=wkvTc[0:128, :])
        nc.sync.dma_start(out=wq_t[0][:], in_=wqTc[0:128, :])
        make_identity(nc, ident[:])

        dram = top.enter_context(tc.tile_pool(name="dram", bufs=1, space="DRAM"))
        a2a_in = [dram.tile([8, 256, 256], bf, tag=f"a2ain{h}", name=f"a2ain{h}") for h in range(2)]
        a2a_out = [dram.tile([8, 256, 256], bf, tag=f"a2aout{h}", name=f"a2aout{h}") for h in range(2)]
        a2aB_in = [dram.tile([8, 128, 256], bf, tag=f"aBin{p}", name=f"aBin{p}") for p in range(2)]
        a2aB_out = [dram.tile([8, 128, 256], bf, tag=f"aBout{p}", name=f"aBout{p}") for p in range(2)]

        xp = top.enter_context(tc.tile_pool(name="xchunk", bufs=16))
        vstage = top.enter_context(tc.tile_pool(name="vstage", bufs=2))
        rtmp = top.enter_context(tc.tile_pool(name="ropetmp", bufs=1))
        esp = top.enter_context(tc.tile_pool(name="expsbuf", bufs=3))
        nrm = top.enter_context(tc.tile_pool(name="normtmp", bufs=2))
        wos = top.enter_context(tc.tile_pool(name="wosbuf", bufs=2))
        # PSUM: 'sp' 2x[128,1024]f32 = 4 banks, 'av' 2x[65,512]f32 = 2 banks,
        # 'sm' 2x 2KB slots = 2 banks (proj passes, v-transposes, denominator
        # broadcasts, and output-projection accumulators rotate through 'sm')
        psp = top.enter_context(tc.tile_pool(name="spsum", bufs=2, space="PSUM"))
        avp = top.enter_context(tc.tile_pool(name="avpsum", bufs=2, space="PSUM"))
        smp = top.enter_context(tc.tile_pool(name="smpsum", bufs=2, space="PSUM"))

        qv = [
            qt[p][:].rearrange("p (b u i) -> p b u i", b=2, u=4)
            for p in range(2)
        ]

        # ------------- attention emission (generator, 1 block per tick) ----
        def scores(b, u, p, kb, tag):
            kcol = 2048 * b + 128 * kb
            sp = psp.tile([128, 1024], f32, tag="sp", name=f"sp{tag}{kb}")
            for hh in range(2):
                r0 = 64 * hh
                nc.tensor.matmul(
                    sp[:, 512 * hh:512 * hh + 512],
                    kt[r0:r0 + 64, kcol:kcol + 128],
                    qv[p][r0:r0 + 64, b, u, :],
                    start=True, stop=True,
                )
            return sp

        def expav(b, u, p, kb, sp, av, first, last, tag):
            ex = esp.tile([128, 1024], bf, tag="ex", name=f"ex{tag}{kb}")
            nc.scalar.activation(ex[:], sp[:], AF.Exp, scale=SCALE)
            if kb >= 4 * u:
                # zero the causally-invalid staircase (cols < 128*(j+1))
                j = kb - 4 * u
                w = 128 * (j + 1)
                for hh in range(2):
                    c0 = 512 * hh
                    nc.vector.tensor_mul(
                        ex[:, c0:c0 + w], ex[:, c0:c0 + w], msk[j][:, 0:w]
                    )
            v_ = vt[16 * b + kb]
            for hh in range(2):
                nc.tensor.matmul(
                    av[hh][:, 0:512], v_[:], ex[:, 512 * hh:512 * hh + 512],
                    start=first, stop=last,
                )

        def make_norm(b, u, p, av, tag):
            def avcopy():
                cps = []
                for hh in range(2):
                    cp = nrm.tile([65, 512], bf, tag="cp", name=f"cp{tag}{hh}")
                    nc.vector.tensor_copy(cp[:], av[hh][0:65, :])
                    cps.append(cp)
                return cps

            def recip(cps):
                rrs = []
                for hh in range(2):
                    bcp = smp.tile([128, 512], f32, tag="sm", name=f"bc{tag}{hh}")
                    nc.tensor.matmul(
                        bcp[0:64, :], ones_t[64:65, 0:64], cps[hh][64:65, :],
                        start=True, stop=True,
                    )
                    rr = nrm.tile([64, 512], f32, tag="rr", name=f"rr{tag}{hh}")
                    nc.vector.reciprocal_approx_fast(rr[:], bcp[0:64, :])
                    rrs.append(rr)
                return rrs

            def rest(cps, rrs):
                for hh in range(2):
                    at_ = nrm.tile([64, 512], bf, tag="at", name=f"at{tag}{hh}")
                    nc.vector.tensor_mul(at_[:], cps[hh][0:64, :], rrs[hh][:])
                    for hf in range(2):
                        dst = 4 * b + 2 * (u % 2) + hf
                        if u // 2 == 0:
                            nc.sync.dma_start(
                                out=a2a_in[0][
                                    dst, 128 * p + 64 * hh:128 * p + 64 * hh + 64, :
                                ],
                                in_=at_[:, 256 * hf:256 * hf + 256],
                            )
                        else:
                            nc.sync.dma_start(
                                out=a2aB_in[p][dst, 64 * hh:64 * hh + 64, :],
                                in_=at_[:, 256 * hf:256 * hf + 256],
                            )

            return avcopy, recip, rest

        def emit_a2a(h):
            nc.gpsimd.collective_compute(
                "AllToAll",
                mybir.AluOpType.bypass,
                replica_groups=[list(range(8))],
                ins=[a2a_in[h][:].opt()],
                outs=[a2a_out[h][:].opt()],
            )
            for e in range(16):
                nc.sync.dma_start(
                    out=rh_t[h][e][:],
                    in_=a2a_out[h][e // 2, 128 * (e % 2):128 * (e % 2) + 128, :],
                )

        def emit_a2aB(p):
            nc.gpsimd.collective_compute(
                "AllToAll",
                mybir.AluOpType.bypass,
                replica_groups=[list(range(8))],
                ins=[a2aB_in[p][:].opt()],
                outs=[a2aB_out[p][:].opt()],
            )
            for s_ in range(8):
                nc.sync.dma_start(
                    out=rh_t[1][2 * s_ + p][:], in_=a2aB_out[p][s_]
                )

        def phase3_unit(h, m):
            po = smp.tile([128, 256], f32, tag="sm", name=f"po{h}{m}")
            # part B: even e-tiles (p=0 rows) arrive in the first half-size
            # AllToAll; contract them first so odd tiles get more cover
            order = list(range(16)) if h == 0 else \
                [0, 2, 4, 6, 8, 10, 12, 14, 1, 3, 5, 7, 9, 11, 13, 15]
            for i_, e in enumerate(order):
                nc.tensor.matmul(
                    po[:], wo_t[e][:, 128 * m:128 * (m + 1)], rh_t[h][e][:],
                    start=(i_ == 0), stop=(i_ == 15),
                )
            os_ = wos.tile([128, 256], f32, tag="os")
            nc.vector.tensor_copy(os_[:], po[:])
            nc.sync.dma_start(
                out=outT[128 * m:128 * (m + 1), 256 * h:256 * h + 256], in_=os_[:]
            )

        # p-group order by causal availability: (0,u<=1) need only C0;
        # (1,u<=1) need C1; (0,u>=2) need C2; (1,u>=2) need C3
        GROUPS = [
            (b, u, p)
            for (b, u) in [(0, 0), (0, 1), (1, 0), (1, 1),
                           (0, 2), (0, 3), (1, 2), (1, 3)]
            for p in range(2)
        ]

        p3 = [0]  # phase-3 part-A units emitted

        def attn_stream():
            pending = {"recip": None, "rest": None}
            a2a0 = [False]
            for gi, (b, u, p) in enumerate(GROUPS):
                tag = f"{b}{u}{p}"
                av = [
                    avp.tile([HD + 1, 512], f32, tag="av", name=f"av{tag}{hh}")
                    for hh in range(2)
                ]
                avcopy, recip, rest = make_norm(b, u, p, av, tag)
                nkb = 4 * u + 4
                order = list(range(4 * u)) + [4 * u + j for j in range(4)]
                pipe = []
                for i, kb in enumerate(order):
                    sp = scores(b, u, p, kb, tag)
                    pipe.append((kb, sp))
                    if i == 1 and pending["recip"] is not None:
                        rrs = pending["recip"]()
                        pending["recip"] = None
                        prev_rest, prev_cps = pending["rest"]
                        pending["rest"] = (lambda pr=prev_rest, pc=prev_cps, r=rrs: pr(pc, r))
                    if i == 3 and callable(pending["rest"]):
                        pending["rest"]()
                        pending["rest"] = None
                        if gi == 8 and not a2a0[0]:
                            # all u in {0,1} attention outputs written
                            emit_a2a(0)
                            a2a0[0] = True
                        elif gi == 15:
                            # all p=0 rests of u in {2,3} written
                            emit_a2aB(0)
                    if len(pipe) > 1:
                        pk, psp_ = pipe.pop(0)
                        expav(b, u, p, pk, psp_, av,
                              first=(pk == order[0]), last=(pk == order[-1]), tag=tag)
                    # phase-3 part A interleaved once the first AllToAll has
                    # had time to complete (from group 10 onward)
                    if gi >= 12 and i % 4 == 2 and p3[0] < 8:
                        phase3_unit(0, p3[0])
                        p3[0] += 1
                    yield
                for pk, psp_ in pipe:
                    expav(b, u, p, pk, psp_, av,
                          first=(pk == order[0]), last=(pk == order[-1]), tag=tag)
                cps = avcopy()
                pending["recip"] = (lambda rc=recip, c=cps: rc(c))
                pending["rest"] = (rest, cps)
                yield
            # final group: flush its normalization immediately
            rrs = pending["recip"]()
            prev_rest, prev_cps = pending["rest"]
            prev_rest(prev_cps, rrs)


        stream = attn_stream()
        ticks_left = [0]

        def tick():
            if ticks_left[0] <= 0:
                return
            try:
                next(stream)
                ticks_left[0] -= 1
            except StopIteration:
                ticks_left[0] = 0

        # ------------- projection chunk (b, half) with interleave ---------
        def emit_x(b, half):
            xt = []
            for d in range(16):
                x_ = xp.tile([128, 1024], bf, tag="x", name=f"x{b}{half}{d}")
                nc.sync.dma_start(out=x_[:], in_=xT[b, d, half])
                xt.append(x_)
            return xt

        def rope_k_v(b, half, s, pkv, col, tp=None):
            # v: copy + PE-transpose 4 key-blocks of 128
            vs = vstage.tile([128, 512], bf, tag="vs")
            nc.vector.tensor_copy(vs[64:128, :], pkv[64:128, :])
            t1 = rtmp.tile([128, 512], f32, tag="t1")
            nc.vector.tensor_mul(t1[0:64, :], pkv[0:64, :], cos_t[0:64, col:col + 512])
            sw = rtmp.tile([128, 512], f32, tag="sw")
            nc.vector.stream_shuffle(sw[0:64, :], pkv[0:64, :], SW)
            t2 = rtmp.tile([128, 512], f32, tag="t2")
            nc.vector.tensor_mul(t2[0:64, :], sw[0:64, :], sin_t[0:64, col:col + 512])
            nc.vector.tensor_add(kt[0:64, col:col + 512], t1[0:64, :], t2[0:64, :])
            nc.sync.dma_start(
                out=kt[64:128, col:col + 512], in_=kt[0:64, col:col + 512]
            )
            for j in range(4):
                if tp is None:
                    ptv = smp.tile([128, HD], bf, tag="sm", name=f"tv{b}{half}{s}{j}")
                else:
                    ptv = tp.tile([128, HD], bf, tag="av", name=f"tv{b}{half}{s}{j}")
                nc.tensor.transpose(
                    ptv[:], vs[64:128, 128 * j:128 * (j + 1)], ident[64:128, 64:128]
                )
                kb = 8 * half + 4 * s + j
                nc.vector.tensor_copy(vt[16 * b + kb][:, 0:HD], ptv[:])
                nc.sync.dma_start(
                    out=vt[16 * b + kb][:, HD:HD + 1], in_=onesd[:, 0:1]
                )

        def rope_q(b, half, s, p, pq, col):
            t1 = rtmp.tile([128, 512], f32, tag="t1")
            nc.vector.tensor_mul(t1[:], pq[:], cos_t[:, col:col + 512])
            sw = rtmp.tile([128, 512], f32, tag="sw")
            nc.vector.stream_shuffle(sw[:], pq[:], SW)
            t2 = rtmp.tile([128, 512], f32, tag="t2")
            nc.vector.tensor_mul(t2[:], sw[:], sin_t[:, col:col + 512])
            nc.vector.tensor_add(qt[p][:, col:col + 512], t1[:], t2[:])

        def proj_chunk0(xt):
            # C0: no attention to interleave; issue all 6 accumulations per
            # arriving chunk so the PE keeps pace with the cold DMA stream
            b = half = 0
            pkv = [smp.tile([128, 512], f32, tag="sm", name=f"c0kv{s}") for s in range(2)]
            pq0 = psp.tile([128, 1024], f32, tag="sp", name="c0q0")
            pq1 = psp.tile([128, 1024], f32, tag="sp", name="c0q1")
            for d in range(16):
                for s in range(2):
                    xs = xt[d][:, 512 * s:512 * s + 512]
                    nc.tensor.matmul(pkv[s][:], wkv_t[d][:], xs,
                                     start=(d == 0), stop=(d == 15))
                    nc.tensor.matmul(pq0[:, 512 * s:512 * s + 512], wq_t[d][:, 0:128],
                                     xs, start=(d == 0), stop=(d == 15))
                    nc.tensor.matmul(pq1[:, 512 * s:512 * s + 512], wq_t[d][:, 128:256],
                                     xs, start=(d == 0), stop=(d == 15))
            for s in range(2):
                rope_k_v(b, half, s, pkv[s], 512 * s, tp=avp)
            for s in range(2):
                rope_q(b, half, s, 0, pq0[:, 512 * s:512 * s + 512], 512 * s)
                rope_q(b, half, s, 1, pq1[:, 512 * s:512 * s + 512], 512 * s)

        def proj_chunk(b, half, nticks, xt=None):
            ticks_left[0] = nticks
            if xt is None:
                xt = emit_x(b, half)
            for s in range(2):
                col = 2048 * b + 1024 * half + 512 * s

                # kv pass
                pkv = smp.tile([128, 512], f32, tag="sm", name=f"pkv{b}{half}{s}")
                for d in range(16):
                    nc.tensor.matmul(
                        pkv[:], wkv_t[d][:], xt[d][:, 512 * s:512 * s + 512],
                        start=(d == 0), stop=(d == 15),
                    )
                    if d % 2 == 1:
                        tick()
                # v: copy + PE-transpose 4 key-blocks of 128
                vs = vstage.tile([128, 512], bf, tag="vs")
                nc.vector.tensor_copy(vs[64:128, :], pkv[64:128, :])
                # k RoPE on rows 0:64, then duplicate to 64:128
                t1 = rtmp.tile([128, 512], f32, tag="t1")
                nc.vector.tensor_mul(t1[0:64, :], pkv[0:64, :], cos_t[0:64, col:col + 512])
                sw = rtmp.tile([128, 512], f32, tag="sw")
                nc.vector.stream_shuffle(sw[0:64, :], pkv[0:64, :], SW)
                t2 = rtmp.tile([128, 512], f32, tag="t2")
                nc.vector.tensor_mul(t2[0:64, :], sw[0:64, :], sin_t[0:64, col:col + 512])
                nc.vector.tensor_add(kt[0:64, col:col + 512], t1[0:64, :], t2[0:64, :])
                nc.sync.dma_start(
                    out=kt[64:128, col:col + 512], in_=kt[0:64, col:col + 512]
                )
                for j in range(4):
                    ptv = smp.tile([128, HD], bf, tag="sm", name=f"tv{b}{half}{s}{j}")
                    nc.tensor.transpose(
                        ptv[:], vs[64:128, 128 * j:128 * (j + 1)], ident[64:128, 64:128]
                    )
                    kb = 8 * half + 4 * s + j
                    nc.vector.tensor_copy(vt[16 * b + kb][:, 0:HD], ptv[:])
                    nc.sync.dma_start(
                        out=vt[16 * b + kb][:, HD:HD + 1], in_=onesd[:, 0:1]
                    )
                tick()

                # q passes (head-pair p = 0, 1)
                for p in range(2):
                    pq = smp.tile([128, 512], f32, tag="sm", name=f"pq{b}{half}{s}{p}")
                    for d in range(16):
                        nc.tensor.matmul(
                            pq[:], wq_t[d][:, 128 * p:128 * p + 128],
                            xt[d][:, 512 * s:512 * s + 512],
                            start=(d == 0), stop=(d == 15),
                        )
                        if d % 4 == 3:
                            tick()
                    t1 = rtmp.tile([128, 512], f32, tag="t1")
                    nc.vector.tensor_mul(t1[:], pq[:], cos_t[:, col:col + 512])
                    sw = rtmp.tile([128, 512], f32, tag="sw")
                    nc.vector.stream_shuffle(sw[:], pq[:], SW)
                    t2 = rtmp.tile([128, 512], f32, tag="t2")
                    nc.vector.tensor_mul(t2[:], sw[:], sin_t[:, col:col + 512])
                    nc.vector.tensor_add(qt[p][:, col:col + 512], t1[:], t2[:])
                    tick()

        # wo loads stream in the background once the first chunk is queued
        def load_wo(lo, hi):
            for e in range(lo, hi):
                nc.sync.dma_start(out=wo_t[e][:], in_=woT[128 * e:128 * (e + 1), :])

        # chunks in causal-availability order; tick budgets:
        # after C0: G(0,0)=8 blocks(+2 transitions); after C1: G(1,0)+G(0,1);
        # after C2: G(1,1)+G(0,2); then drain
        xt0 = []
        for d in range(16):
            if d >= 1:
                nc.sync.dma_start(out=wkv_t[d][:], in_=wkvTc[128 * d:128 * (d + 1), :])
                nc.sync.dma_start(out=wq_t[d][:], in_=wqTc[128 * d:128 * (d + 1), :])
            x_ = xp.tile([128, 1024], bf, tag="x", name=f"x00{d}")
            nc.sync.dma_start(out=x_[:], in_=xT[0, d, 0])
            xt0.append(x_)
        nc.sync.dma_start(out=cos_t[:, 0:S], in_=cosd[:])
        nc.sync.dma_start(out=sin_t[:, 0:S], in_=sind[:])
        nc.sync.dma_start(out=cos_t[:, S:2 * S], in_=cos_t[:, 0:S])
        nc.sync.dma_start(out=sin_t[:, S:2 * S], in_=sin_t[:, 0:S])
        for j in range(4):
            nc.sync.dma_start(out=msk[j][:], in_=maskd[j])
        proj_chunk0(xt0)
        proj_chunk(1, 0, 28)
        load_wo(0, 8)
        proj_chunk(0, 1, 54)
        load_wo(8, 16)
        proj_chunk(1, 1, 34)
        # drain the rest of attention + interleaved phase-3 part A
        ticks_left[0] = 10 ** 9
        for _ in stream:
            pass

        # ---------------- tail: second AllToAll + part B -------------------
        emit_a2aB(1)
        # cover the AllToAll wait with the reserved part-A units + warm junk
        for m in range(p3[0], 16):
            phase3_unit(0, m)
        warm = smp.tile([128, 256], f32, tag="sm", name="warmbank")
        for w in range(12):
            nc.tensor.matmul(
                warm[:], wo_t[0][:, 0:128], rh_t[0][0][:],
                start=True, stop=True, skip_group_check=True,
            )
        for m in range(16):
            phase3_unit(1, m)

    nc.compile()
    _BUILT = nc
    return nc


def _host_inputs(x, wq, wk, wv, wo):
    """Per-core input maps (host-side layout prep only, no math on x)."""
    import ml_dtypes

    bf16 = ml_dtypes.bfloat16
    x = np.ascontiguousarray(x, dtype=np.float32)
    xT3 = x.transpose(0, 2, 1)
    xT = np.ascontiguousarray(
        xT3.reshape(B, 16, 128, 2, 1024).transpose(0, 1, 3, 2, 4).astype(bf16)
    )
    woT = np.ascontiguousarray(np.asarray(wo, np.float32).T.astype(bf16))

    inv = THETA ** (-np.arange(32, dtype=np.float64) / 32.0)
    ang = np.outer(inv, np.arange(S, dtype=np.float64))  # [32, S]
    cos1 = np.cos(ang).astype(np.float32)
    sin1 = np.sin(ang).astype(np.float32)
    pairs = (np.arange(128) % 64) // 2
    signs = np.where(np.arange(128) % 2 == 0, -1.0, 1.0).astype(np.float32)
    cosd = np.ascontiguousarray(cos1[pairs].astype(bf16))
    sind = np.ascontiguousarray((sin1[pairs] * signs[:, None]).astype(bf16))

    k_i = np.arange(128)[:, None]
    q_i = np.arange(512)[None, :]
    maskd = np.stack(
        [np.where(q_i >= k_i + 128 * j, 1.0, 0.0) for j in range(4)]
    ).astype(bf16)
    onesd = np.ones((128, 64), bf16)

    wq = np.asarray(wq, np.float32)
    wk = np.asarray(wk, np.float32)
    wv = np.asarray(wv, np.float32)
    in_maps = []
    for c in range(NCORES):
        wqTc = np.ascontiguousarray(wq[256 * c:256 * (c + 1), :].T.astype(bf16))
        wkvTc = np.ascontiguousarray(
            np.concatenate(
                [wk[64 * c:64 * (c + 1), :].T, wv[64 * c:64 * (c + 1), :].T], axis=1
            ).astype(bf16)
        )
        in_maps.append(
            {
                "xT": xT, "wqTc": wqTc, "wkvTc": wkvTc, "woT": woT,
                "cosd": cosd, "sind": sind, "maskd": maskd, "onesd": onesd,
            }
        )
    return in_maps


def run(x, wq, wk, wv, wo, trace=False):
    """Build, run on 8 cores, assemble full output. Returns (out, results)."""
    from concourse.bass_utils import run_bass_kernel_spmd

    nc = _build()
    in_maps = _host_inputs(x, wq, wk, wv, wo)
    r = run_bass_kernel_spmd(nc, in_maps, list(range(NCORES)), trace=trace)
    out = np.empty((B, S, D), np.float32)
    for c in range(NCORES):
        b, j = c // 4, c % 4
        oT = r.results[c]["outT"]
        qa = 512 * (j // 2) + 256 * (j % 2)
        qb = 1024 + 512 * (j // 2) + 256 * (j % 2)
        out[b, qa:qa + 256, :] = oT[:, 0:256].T
        out[b, qb:qb + 256, :] = oT[:, 256:512].T
    return out, r


def kernel(x, wq, wk, wv, wo):
    out, _ = run(x, wq, wk, wv, wo, trace=False)
    return out


# revision 18
# speedup vs baseline: 1.0689x; 1.0689x over previous
"""Trainium2 Bass kernel: GQA attention (B=2, S=2048, D=2048, 32 q-heads,
8 kv-heads, head_dim 64, RoPE interleaved, causal) on 8 NeuronCores.

Sharding: tensor-parallel over heads. Core c owns q-heads 4c..4c+3 (= kv head
c) for BOTH batch elements. The kernel streams x one (batch, half) chunk at a
time, projecting q/k/v for that chunk, and INTERLEAVES causal-attention block
work for already-projected query groups between the projection passes so the
PE never idles (idle PE re-throttles the HAM clock gate to 1.2 GHz). Query
groups are contiguous 512-token quarters; group (b,u) needs key blocks
0..4u+3, so groups become runnable as soon as their (batch, half) chunks are
projected. Attention outputs for u in {0,1} ship through an 8-core AllToAll
mid-kernel while u in {2,3} attention still runs; the output projection for
the first token-halves overlaps the remaining attention, and only the second
AllToAll + final 16 matmul chains sit in the tail.

All matmul inputs are bf16 (PSUM accumulation stays fp32). exp runs on
ScalarE with the 1/sqrt(hd) scale folded into the activation's affine
pre-scale. Softmax denominators come from a ones-column accumulated alongside
attn@V; normalization broadcasts the raw sum with a K=1 matmul FIRST, then
reciprocals the [64,512] broadcast (a [1,512] reciprocal serializes one DVE
lane at 3.3us).
"""

import numpy as np

B, S, D = 2, 2048, 2048
NH, NKV, HD = 32, 8, 64
THETA = 10000.0
NCORES = 8
NEG = -1.0e30

_BUILT = None


def _swap_mask():
    m = []
    for i in range(16):
        m += [2 * i + 1, 2 * i]
    return m


def _build():
    """Build + compile the SPMD Bass program (once per process)."""
    global _BUILT
    if _BUILT is not None:
        return _BUILT

    from contextlib import ExitStack

    import concourse.tile as tile
    from concourse import bacc, mybir
    from concourse.masks import make_identity

    f32 = mybir.dt.float32
    bf = mybir.dt.bfloat16
    AF = mybir.ActivationFunctionType

    nc = bacc.Bacc(
        "TRN2", target_bir_lowering=False, debug=False, num_devices=NCORES
    )

    xT = nc.dram_tensor("xT", [B, 16, 2, 128, 1024], bf, kind="ExternalInput").ap()
    wqTc = nc.dram_tensor("wqTc", [D, 256], bf, kind="ExternalInput").ap()
    wkvTc = nc.dram_tensor("wkvTc", [D, 128], bf, kind="ExternalInput").ap()
    woT = nc.dram_tensor("woT", [D, D], bf, kind="ExternalInput").ap()
    cosd = nc.dram_tensor("cosd", [128, S], bf, kind="ExternalInput").ap()
    sind = nc.dram_tensor("sind", [128, S], bf, kind="ExternalInput").ap()
    maskd = nc.dram_tensor("maskd", [4, 128, 512], bf, kind="ExternalInput").ap()
    onesd = nc.dram_tensor("onesd", [128, 64], bf, kind="ExternalInput").ap()
    outT = nc.dram_tensor("outT", [D, 512], f32, kind="ExternalOutput").ap()

    SW = _swap_mask()
    SCALE = 1.0 / float(np.sqrt(HD))

    with tile.TileContext(nc) as tc, ExitStack() as top:
        top.enter_context(
            nc.allow_low_precision(reason="bf16 matmul inputs by design")
        )
        res = top.enter_context(tc.tile_pool(name="resident", bufs=1))
        qt = [res.tile([128, B * S], bf, tag=f"qt{p}", name=f"qt{p}") for p in range(2)]
        kt = res.tile([128, B * S], bf, tag="kt")  # kv head, duplicated rows
        vt = [res.tile([128, HD + 1], bf, tag=f"vt{i}", name=f"vt{i}") for i in range(2 * 16)]
        msk = [res.tile([128, 512], bf, tag=f"m{j}", name=f"m{j}") for j in range(4)]
        ones_t = res.tile([128, 64], bf, tag="ones")
        cos_t = res.tile([128, B * S], bf, tag="cos")
        sin_t = res.tile([128, B * S], bf, tag="sin")
        ident = res.tile([128, 128], bf, tag="ident")
        wq_t = [res.tile([128, 256], bf, tag=f"wq{d}", name=f"wq{d}") for d in range(16)]
        wkv_t = [res.tile([128, 128], bf, tag=f"wkv{d}", name=f"wkv{d}") for d in range(16)]
        wo_t = [res.tile([128, D], bf, tag=f"wo{e}", name=f"wo{e}") for e in range(16)]
        rh_t = [
            [res.tile([128, 256], bf, tag=f"rh{h}{e}", name=f"rh{h}{e}") for e in range(16)]
            for h in range(2)
        ]

        # weight + first-chunk loads first so the first matmul starts early
        nc.sync.dma_start(out=ones_t[:], in_=onesd[:])
        nc.sync.dma_start(out=wkv_t[0][:], in_=wkvTc[0:128, :])
        nc.sync.dma_start(out=wq_t[0][:], in_=wqTc[0:128, :])
        make_identity(nc, ident[:])

        dram = top.enter_context(tc.tile_pool(name="dram", bufs=1, space="DRAM"))
        a2a_in = [dram.tile([8, 256, 256], bf, tag=f"a2ain{h}", name=f"a2ain{h}") for h in range(2)]
        a2a_out = [dram.tile([8, 256, 256], bf, tag=f"a2aout{h}", name=f"a2aout{h}") for h in range(2)]
        a2aB_in = [dram.tile([8, 128, 256], bf, tag=f"aBin{p}", name=f"aBin{p}") for p in range(2)]
        a2aB_out = [dram.tile([8, 128, 256], bf, tag=f"aBout{p}", name=f"aBout{p}") for p in range(2)]

        xp = top.enter_context(tc.tile_pool(name="xchunk", bufs=16))
        vstage = top.enter_context(tc.tile_pool(name="vstage", bufs=2))
        rtmp = top.enter_context(tc.tile_pool(name="ropetmp", bufs=1))
        esp = top.enter_context(tc.tile_pool(name="expsbuf", bufs=3))
        nrm = top.enter_context(tc.tile_pool(name="normtmp", bufs=2))
        wos = top.enter_context(tc.tile_pool(name="wosbuf", bufs=2))
        # PSUM: 'sp' 2x[128,1024]f32 = 4 banks, 'av' 2x[65,512]f32 = 2 banks,
        # 'sm' 2x 2KB slots = 2 banks (proj passes, v-transposes, denominator
        # broadcasts, and output-projection accumulators rotate through 'sm')
        psp = top.enter_context(tc.tile_pool(name="spsum", bufs=2, space="PSUM"))
        avp = top.enter_context(tc.tile_pool(name="avpsum", bufs=2, space="PSUM"))
        smp = top.enter_context(tc.tile_pool(name="smpsum", bufs=2, space="PSUM"))

        qv = [
            qt[p][:].rearrange("p (b u i) -> p b u i", b=2, u=4)
            for p in range(2)
        ]

        # ------------- attention emission (generator, 1 block per tick) ----
        def scores(b, u, p, kb, tag):
            kcol = 2048 * b + 128 * kb
            sp = psp.tile([128, 1024], f32, tag="sp", name=f"sp{tag}{kb}")
            for hh in range(2):
                r0 = 64 * hh
                nc.tensor.matmul(
                    sp[:, 512 * hh:512 * hh + 512],
                    kt[r0:r0 + 64, kcol:kcol + 128],
                    qv[p][r0:r0 + 64, b, u, :],
                    start=True, stop=True,
                )
            return sp

        def expav(b, u, p, kb, sp, av, first, last, tag):
            ex = esp.tile([128, 1024], bf, tag="ex", name=f"ex{tag}{kb}")
            j = kb - 4 * u  # >= 0 on the diagonal
            if j >= 2:
                # deep diag: skip exp on the all-invalid prefix; zero it on
                # the idle GpSimd engine, exp only the valid column suffix
                c = 128 * j
                for hh in range(2):
                    nc.gpsimd.memset(ex[:, 512 * hh:512 * hh + c], 0.0)
                spv = sp[:].rearrange("p (h i) -> p h i", h=2)[:, :, c:512]
                exv = ex[:].rearrange("p (h i) -> p h i", h=2)[:, :, c:512]
                nc.scalar.activation(exv, spv, AF.Exp, scale=SCALE)
                for hh in range(2):
                    c0 = 512 * hh
                    nc.vector.tensor_mul(
                        ex[:, c0 + c:c0 + c + 128], ex[:, c0 + c:c0 + c + 128],
                        msk[j][:, c:c + 128]
                    )
            else:
                nc.scalar.activation(ex[:], sp[:], AF.Exp, scale=SCALE)
                if j >= 0:
                    # zero the causally-invalid staircase (cols < 128*(j+1))
                    w = 128 * (j + 1)
                    for hh in range(2):
                        c0 = 512 * hh
                        nc.vector.tensor_mul(
                            ex[:, c0:c0 + w], ex[:, c0:c0 + w], msk[j][:, 0:w]
                        )
            v_ = vt[16 * b + kb]
            for hh in range(2):
                nc.tensor.matmul(
                    av[hh][:, 0:512], v_[:], ex[:, 512 * hh:512 * hh + 512],
                    start=first, stop=last,
                )

        def make_norm(b, u, p, av, tag):
            def avcopy():
                cps = []
                for hh in range(2):
                    cp = nrm.tile([65, 512], bf, tag="cp", name=f"cp{tag}{hh}")
                    nc.vector.tensor_copy(cp[:], av[hh][0:65, :])
                    cps.append(cp)
                return cps

            def recip(cps):
                rrs = []
                for hh in range(2):
                    bcp = smp.tile([128, 512], f32, tag="sm", name=f"bc{tag}{hh}")
                    nc.tensor.matmul(
                        bcp[0:64, :], ones_t[64:65, 0:64], cps[hh][64:65, :],
                        start=True, stop=True,
                    )
                    rr = nrm.tile([64, 512], f32, tag="rr", name=f"rr{tag}{hh}")
                    nc.vector.reciprocal_approx_fast(rr[:], bcp[0:64, :])
                    rrs.append(rr)
                return rrs

            def rest(cps, rrs):
                for hh in range(2):
                    at_ = nrm.tile([64, 512], bf, tag="at", name=f"at{tag}{hh}")
                    nc.vector.tensor_mul(at_[:], cps[hh][0:64, :], rrs[hh][:])
                    for hf in range(2):
                        dst = 4 * b + 2 * (u % 2) + hf
                        if u // 2 == 0:
                            nc.sync.dma_start(
                                out=a2a_in[0][
                                    dst, 128 * p + 64 * hh:128 * p + 64 * hh + 64, :
                                ],
                                in_=at_[:, 256 * hf:256 * hf + 256],
                            )
                        else:
                            nc.sync.dma_start(
                                out=a2aB_in[p][dst, 64 * hh:64 * hh + 64, :],
                                in_=at_[:, 256 * hf:256 * hf + 256],
                            )

            return avcopy, recip, rest

        def emit_a2a(h):
            nc.gpsimd.collective_compute(
                "AllToAll",
                mybir.AluOpType.bypass,
                replica_groups=[list(range(8))],
                ins=[a2a_in[h][:].opt()],
                outs=[a2a_out[h][:].opt()],
            )
            for e in range(16):
                nc.sync.dma_start(
                    out=rh_t[h][e][:],
                    in_=a2a_out[h][e // 2, 128 * (e % 2):128 * (e % 2) + 128, :],
                )

        def emit_a2aB(p):
            nc.gpsimd.collective_compute(
                "AllToAll",
                mybir.AluOpType.bypass,
                replica_groups=[list(range(8))],
                ins=[a2aB_in[p][:].opt()],
                outs=[a2aB_out[p][:].opt()],
            )
            for s_ in range(8):
                nc.sync.dma_start(
                    out=rh_t[1][2 * s_ + p][:], in_=a2aB_out[p][s_]
                )

        def phase3_unit(h, m):
            po = smp.tile([128, 256], f32, tag="sm", name=f"po{h}{m}")
            # part B: even e-tiles (p=0 rows) arrive in the first half-size
            # AllToAll; contract them first so odd tiles get more cover
            order = list(range(16)) if h == 0 else \
                [0, 2, 4, 6, 8, 10, 12, 14, 1, 3, 5, 7, 9, 11, 13, 15]
            for i_, e in enumerate(order):
                nc.tensor.matmul(
                    po[:], wo_t[e][:, 128 * m:128 * (m + 1)], rh_t[h][e][:],
                    start=(i_ == 0), stop=(i_ == 15),
                )
            os_ = wos.tile([128, 256], f32, tag="os")
            nc.vector.tensor_copy(os_[:], po[:])
            nc.sync.dma_start(
                out=outT[128 * m:128 * (m + 1), 256 * h:256 * h + 256], in_=os_[:]
            )

        # p-group order by causal availability: (0,u<=1) need only C0;
        # (1,u<=1) need C1; (0,u>=2) need C2; (1,u>=2) need C3
        GROUPS = [
            (b, u, p)
            for (b, u) in [(0, 0), (0, 1), (1, 0), (1, 1),
                           (0, 2), (0, 3), (1, 2), (1, 3)]
            for p in range(2)
        ]

        p3 = [0]  # phase-3 part-A units emitted

        def attn_stream():
            pending = {"recip": None, "rest": None}
            a2a0 = [False]
            for gi, (b, u, p) in enumerate(GROUPS):
                tag = f"{b}{u}{p}"
                av = [
                    avp.tile([HD + 1, 512], f32, tag="av", name=f"av{tag}{hh}")
                    for hh in range(2)
                ]
                avcopy, recip, rest = make_norm(b, u, p, av, tag)
                nkb = 4 * u + 4
                order = list(range(4 * u)) + [4 * u + j for j in range(4)]
                pipe = []
                for i, kb in enumerate(order):
                    sp = scores(b, u, p, kb, tag)
                    pipe.append((kb, sp))
                    if i == 1 and pending["recip"] is not None:
                        rrs = pending["recip"]()
                        pending["recip"] = None
                        prev_rest, prev_cps = pending["rest"]
                        pending["rest"] = (lambda pr=prev_rest, pc=prev_cps, r=rrs: pr(pc, r))
                    if i == 3 and callable(pending["rest"]):
                        pending["rest"]()
                        pending["rest"] = None
                        if gi == 8 and not a2a0[0]:
                            # all u in {0,1} attention outputs written
                            emit_a2a(0)
                            a2a0[0] = True
                        elif gi == 15:
                            # all p=0 rests of u in {2,3} written
                            emit_a2aB(0)
                    if len(pipe) > 1:
                        pk, psp_ = pipe.pop(0)
                        expav(b, u, p, pk, psp_, av,
                              first=(pk == order[0]), last=(pk == order[-1]), tag=tag)
                    # phase-3 part A interleaved once the first AllToAll has
                    # had time to complete (from group 10 onward)
                    if gi >= 12 and i % 4 == 2 and p3[0] < 8:
                        phase3_unit(0, p3[0])
                        p3[0] += 1
                    yield
                for pk, psp_ in pipe:
                    expav(b, u, p, pk, psp_, av,
                          first=(pk == order[0]), last=(pk == order[-1]), tag=tag)
                cps = avcopy()
                pending["recip"] = (lambda rc=recip, c=cps: rc(c))
                pending["rest"] = (rest, cps)
                yield
            # final group: flush its normalization immediately
            rrs = pending["recip"]()
            prev_rest, prev_cps = pending["rest"]
            prev_rest(prev_cps, rrs)


        stream = attn_stream()
        ticks_left = [0]

        def tick():
            if ticks_left[0] <= 0:
                return
            try:
                next(stream)
                ticks_left[0] -= 1
            except StopIteration:
                ticks_left[0] = 0

        # ------------- projection chunk (b, half) with interleave ---------
        def emit_x(b, half):
            xt = []
            for d in range(16):
                x_ = xp.tile([128, 1024], bf, tag="x", name=f"x{b}{half}{d}")
                nc.sync.dma_start(out=x_[:], in_=xT[b, d, half])
                xt.append(x_)
            return xt

        def rope_k_v(b, half, s, pkv, col, tp=None):
            # v: copy + PE-transpose 4 key-blocks of 128
            vs = vstage.tile([128, 512], bf, tag="vs")
            nc.vector.tensor_copy(vs[64:128, :], pkv[64:128, :])
            t1 = rtmp.tile([128, 512], f32, tag="t1")
            nc.vector.tensor_mul(t1[0:64, :], pkv[0:64, :], cos_t[0:64, col:col + 512])
            sw = rtmp.tile([128, 512], f32, tag="sw")
            nc.vector.stream_shuffle(sw[0:64, :], pkv[0:64, :], SW)
            t2 = rtmp.tile([128, 512], f32, tag="t2")
            nc.vector.tensor_mul(t2[0:64, :], sw[0:64, :], sin_t[0:64, col:col + 512])
            nc.vector.tensor_add(kt[0:64, col:col + 512], t1[0:64, :], t2[0:64, :])
            nc.sync.dma_start(
                out=kt[64:128, col:col + 512], in_=kt[0:64, col:col + 512]
            )
            for j in range(4):
                if tp is None:
                    ptv = smp.tile([128, HD], bf, tag="sm", name=f"tv{b}{half}{s}{j}")
                else:
                    ptv = tp.tile([128, HD], bf, tag="av", name=f"tv{b}{half}{s}{j}")
                nc.tensor.transpose(
                    ptv[:], vs[64:128, 128 * j:128 * (j + 1)], ident[64:128, 64:128]
                )
                kb = 8 * half + 4 * s + j
                nc.vector.tensor_copy(vt[16 * b + kb][:, 0:HD], ptv[:])
                nc.sync.dma_start(
                    out=vt[16 * b + kb][:, HD:HD + 1], in_=onesd[:, 0:1]
                )

        def rope_q(b, half, s, p, pq, col):
            t1 = rtmp.tile([128, 512], f32, tag="t1")
            nc.vector.tensor_mul(t1[:], pq[:], cos_t[:, col:col + 512])
            sw = rtmp.tile([128, 512], f32, tag="sw")
            nc.vector.stream_shuffle(sw[:], pq[:], SW)
            t2 = rtmp.tile([128, 512], f32, tag="t2")
            nc.vector.tensor_mul(t2[:], sw[:], sin_t[:, col:col + 512])
            nc.vector.tensor_add(qt[p][:, col:col + 512], t1[:], t2[:])

        def proj_chunk0(xt):
            # C0: no attention to interleave; issue all 6 accumulations per
            # arriving chunk so the PE keeps pace with the cold DMA stream
            b = half = 0
            pkv = [smp.tile([128, 512], f32, tag="sm", name=f"c0kv{s}") for s in range(2)]
            pq0 = psp.tile([128, 1024], f32, tag="sp", name="c0q0")
            pq1 = psp.tile([128, 1024], f32, tag="sp", name="c0q1")
            for d in range(16):
                for s in range(2):
                    xs = xt[d][:, 512 * s:512 * s + 512]
                    nc.tensor.matmul(pkv[s][:], wkv_t[d][:], xs,
                                     start=(d == 0), stop=(d == 15))
                    nc.tensor.matmul(pq0[:, 512 * s:512 * s + 512], wq_t[d][:, 0:128],
                                     xs, start=(d == 0), stop=(d == 15))
                    nc.tensor.matmul(pq1[:, 512 * s:512 * s + 512], wq_t[d][:, 128:256],
                                     xs, start=(d == 0), stop=(d == 15))
            for s in range(2):
                rope_k_v(b, half, s, pkv[s], 512 * s, tp=avp)
            for s in range(2):
                rope_q(b, half, s, 0, pq0[:, 512 * s:512 * s + 512], 512 * s)
                rope_q(b, half, s, 1, pq1[:, 512 * s:512 * s + 512], 512 * s)

        def proj_chunk(b, half, nticks, xt=None):
            ticks_left[0] = nticks
            if xt is None:
                xt = emit_x(b, half)
            for s in range(2):
                col = 2048 * b + 1024 * half + 512 * s

                # kv pass
                pkv = smp.tile([128, 512], f32, tag="sm", name=f"pkv{b}{half}{s}")
                for d in range(16):
                    nc.tensor.matmul(
                        pkv[:], wkv_t[d][:], xt[d][:, 512 * s:512 * s + 512],
                        start=(d == 0), stop=(d == 15),
                    )
                    if d % 2 == 1:
                        tick()
                # v: copy + PE-transpose 4 key-blocks of 128
                vs = vstage.tile([128, 512], bf, tag="vs")
                nc.vector.tensor_copy(vs[64:128, :], pkv[64:128, :])
                # k RoPE on rows 0:64, then duplicate to 64:128
                t1 = rtmp.tile([128, 512], f32, tag="t1")
                nc.vector.tensor_mul(t1[0:64, :], pkv[0:64, :], cos_t[0:64, col:col + 512])
                sw = rtmp.tile([128, 512], f32, tag="sw")
                nc.vector.stream_shuffle(sw[0:64, :], pkv[0:64, :], SW)
                t2 = rtmp.tile([128, 512], f32, tag="t2")
                nc.vector.tensor_mul(t2[0:64, :], sw[0:64, :], sin_t[0:64, col:col + 512])
                nc.vector.tensor_add(kt[0:64, col:col + 512], t1[0:64, :], t2[0:64, :])
                nc.sync.dma_start(
                    out=kt[64:128, col:col + 512], in_=kt[0:64, col:col + 512]
                )
                for j in range(4):
                    ptv = smp.tile([128, HD], bf, tag="sm", name=f"tv{b}{half}{s}{j}")
                    nc.tensor.transpose(
                        ptv[:], vs[64:128, 128 * j:128 * (j + 1)], ident[64:128, 64:128]
                    )
                    kb = 8 * half + 4 * s + j
                    nc.vector.tensor_copy(vt[16 * b + kb][:, 0:HD], ptv[:])
                    nc.sync.dma_start(
                        out=vt[16 * b + kb][:, HD:HD + 1], in_=onesd[:, 0:1]
                    )
                tick()

                # q passes (head-pair p = 0, 1)
                for p in range(2):
                    pq = smp.tile([128, 512], f32, tag="sm", name=f"pq{b}{half}{s}{p}")
                    for d in range(16):
                        nc.tensor.matmul(
                            pq[:], wq_t[d][:, 128 * p:128 * p + 128],
                            xt[d][:, 512 * s:512 * s + 512],
                            start=(d == 0), stop=(d == 15),
                        )
                        if d % 4 == 3:
                            tick()
                    t1 = rtmp.tile([128, 512], f32, tag="t1")
                    nc.vector.tensor_mul(t1[:], pq[:], cos_t[:, col:col + 512])
                    sw = rtmp.tile([128, 512], f32, tag="sw")
                    nc.vector.stream_shuffle(sw[:], pq[:], SW)
                    t2 = rtmp.tile([128, 512], f32, tag="t2")
                    nc.vector.tensor_mul(t2[:], sw[:], sin_t[:, col:col + 512])
                    nc.vector.tensor_add(qt[p][:, col:col + 512], t1[:], t2[:])
                    tick()

        # wo loads stream in the background once the first chunk is queued
        def load_wo(lo, hi):
            for e in range(lo, hi):
                nc.sync.dma_start(out=wo_t[e][:], in_=woT[128 * e:128 * (e + 1), :])

        # chunks in causal-availability order; tick budgets:
        # after C0: G(0,0)=8 blocks(+2 transitions); after C1: G(1,0)+G(0,1);
        # after C2: G(1,1)+G(0,2); then drain
        xt0 = []
        for d in range(16):
            if d >= 1:
                nc.sync.dma_start(out=wkv_t[d][:], in_=wkvTc[128 * d:128 * (d + 1), :])
                nc.sync.dma_start(out=wq_t[d][:], in_=wqTc[128 * d:128 * (d + 1), :])
            x_ = xp.tile([128, 1024], bf, tag="x", name=f"x00{d}")
            nc.sync.dma_start(out=x_[:], in_=xT[0, d, 0])
            xt0.append(x_)
        nc.sync.dma_start(out=cos_t[:, 0:S], in_=cosd[:])
        nc.sync.dma_start(out=sin_t[:, 0:S], in_=sind[:])
        nc.sync.dma_start(out=cos_t[:, S:2 * S], in_=cos_t[:, 0:S])
        nc.sync.dma_start(out=sin_t[:, S:2 * S], in_=sin_t[:, 0:S])
        for j in range(4):
            nc.sync.dma_start(out=msk[j][:], in_=maskd[j])
        proj_chunk0(xt0)
        proj_chunk(1, 0, 28)
        load_wo(0, 8)
        proj_chunk(0, 1, 54)
        load_wo(8, 16)
        proj_chunk(1, 1, 34)
        # drain the rest of attention + interleaved phase-3 part A
        ticks_left[0] = 10 ** 9
        for _ in stream:
            pass

        # ---------------- tail: second AllToAll + part B -------------------
        emit_a2aB(1)
        # cover the AllToAll wait with the reserved part-A units + warm junk
        for m in range(p3[0], 16):
            phase3_unit(0, m)
        warm = smp.tile([128, 256], f32, tag="sm", name="warmbank")
        for w in range(12):
            nc.tensor.matmul(
                warm[:], wo_t[0][:, 0:128], rh_t[0][0][:],
                start=True, stop=True, skip_group_check=True,
            )
        for m in range(16):
            phase3_unit(1, m)

    nc.compile()
    _BUILT = nc
    return nc


def _host_inputs(x, wq, wk, wv, wo):
    """Per-core input maps (host-side layout prep only, no math on x)."""
    import ml_dtypes

    bf16 = ml_dtypes.bfloat16
    x = np.ascontiguousarray(x, dtype=np.float32)
    xT3 = x.transpose(0, 2, 1)
    xT = np.ascontiguousarray(
        xT3.reshape(B, 16, 128, 2, 1024).transpose(0, 1, 3, 2, 4).astype(bf16)
    )
    woT = np.ascontiguousarray(np.asarray(wo, np.float32).T.astype(bf16))

    inv = THETA ** (-np.arange(32, dtype=np.float64) / 32.0)
    ang = np.outer(inv, np.arange(S, dtype=np.float64))  # [32, S]
    cos1 = np.cos(ang).astype(np.float32)
    sin1 = np.sin(ang).astype(np.float32)
    pairs = (np.arange(128) % 64) // 2
    signs = np.where(np.arange(128) % 2 == 0, -1.0, 1.0).astype(np.float32)
    cosd = np.ascontiguousarray(cos1[pairs].astype(bf16))
    sind = np.ascontiguousarray((sin1[pairs] * signs[:, None]).astype(bf16))

    k_i = np.arange(128)[:, None]
    q_i = np.arange(512)[None, :]
    maskd = np.stack(
        [np.where(q_i >= k_i + 128 * j, 1.0, 0.0) for j in range(4)]
    ).astype(bf16)
    onesd = np.ones((128, 64), bf16)

    wq = np.asarray(wq, np.float32)
    wk = np.asarray(wk, np.float32)
    wv = np.asarray(wv, np.float32)
    in_maps = []
    for c in range(NCORES):
        wqTc = np.ascontiguousarray(wq[256 * c:256 * (c + 1), :].T.astype(bf16))
        wkvTc = np.ascontiguousarray(
            np.concatenate(
                [wk[64 * c:64 * (c + 1), :].T, wv[64 * c:64 * (c + 1), :].T], axis=1
            ).astype(bf16)
        )
        in_maps.append(
            {
                "xT": xT, "wqTc": wqTc, "wkvTc": wkvTc, "woT": woT,
                "cosd": cosd, "sind": sind, "maskd": maskd, "onesd": onesd,
            }
        )
    return in_maps


def run(x, wq, wk, wv, wo, trace=False):
    """Build, run on 8 cores, assemble full output. Returns (out, results)."""
    from concourse.bass_utils import run_bass_kernel_spmd

    nc = _build()
    in_maps = _host_inputs(x, wq, wk, wv, wo)
    r = run_bass_kernel_spmd(nc, in_maps, list(range(NCORES)), trace=trace)
    out = np.empty((B, S, D), np.float32)
    for c in range(NCORES):
        b, j = c // 4, c % 4
        oT = r.results[c]["outT"]
        qa = 512 * (j // 2) + 256 * (j % 2)
        qb = 1024 + 512 * (j // 2) + 256 * (j % 2)
        out[b, qa:qa + 256, :] = oT[:, 0:256].T
        out[b, qb:qb + 256, :] = oT[:, 256:512].T
    return out, r


def kernel(x, wq, wk, wv, wo):
    out, _ = run(x, wq, wk, wv, wo, trace=False)
    return out
